# revision 2
# baseline (speedup 1.0000x reference)
"""Trainium2 Bass kernel for nn_MultiHeadAttention_52261162058330.

Reference computes, per (batch, head):
    scores = X @ X.T          # [T, T]
    out    = scores @ X       # [T, D]
with X = x[b, h] of shape [T=2048, D=64], no softmax / no scaling.

Key optimization: associativity.  out = (X X^T) X = X (X^T X) = X @ G with
G = X^T X a tiny [64, 64] Gram matrix.  This cuts FLOPs ~32x and is exact
in fp32 up to summation order.

Sharding: B*H = 32 (batch, head) pairs -> 4 heads per core on 8 cores,
fully independent (no collectives).

Per-head dataflow on each core:
  1. DMA X into SBUF as [128, 16, 64]; T is split as (p u) -> partition p
     holds rows 16p..16p+15 (contiguous 4 KiB per partition for the DMA).
  2. TensorE transposes build X^T tiles in PSUM -> copied to SBUF.
  3. 16 accumulating fp32 matmuls: G = sum_u X_u^T X_u   (PSUM [64, 64]).
  4. 16 fp32 matmuls: out rows = (X^T tile).T @ G        (PSUM [128, 64]).
  5. PSUM -> SBUF copies (split over DVE / ACT), DMA out.
"""

import numpy as np

N_CORES = 8
B, H, T, D = 2, 16, 2048, 64
HPC = (B * H) // N_CORES  # heads per core
U = T // 128              # 16 row-tiles per head

_NC = None


def _build():
    import concourse.bacc as bacc
    import concourse.mybir as mybir
    from concourse import tile, masks

    nc = bacc.Bacc(
        trn_type="TRN2", target_bir_lowering=False, debug=False,
        num_devices=N_CORES,
    )
    f32 = mybir.dt.float32
    x_in = nc.dram_tensor("x_shard", [HPC, T, D], f32, kind="ExternalInput").ap()
    y_out = nc.dram_tensor("out_shard", [HPC, T, D], f32, kind="ExternalOutput").ap()
    # T split as (p u): partition p <- rows 16p..16p+15 (contiguous per partition)
    xv = x_in.rearrange("h (p u) d -> p h u d", p=128)
    yv = y_out.rearrange("h (p u) d -> p h u d", p=128)

    with tile.TileContext(nc) as tc:
        with (
            tc.tile_pool(name="const", bufs=1) as cpool,
            tc.tile_pool(name="io", bufs=2) as io,
            tc.tile_pool(name="psT", bufs=2, space="PSUM") as psT,
            tc.tile_pool(name="psG", bufs=2, space="PSUM") as psG,
            tc.tile_pool(name="psO", bufs=2, space="PSUM") as psO,
        ):
            ident = cpool.tile([128, 128], f32)
            masks.make_identity(nc, ident[:])

            for h in range(HPC):
                xsb = io.tile([128, U, D], f32, tag="xsb")
                nc.sync.dma_start(out=xsb[:], in_=xv[:, h])

                # X^T: xt[:, 128u + p] = X[16p + u, :]  (a permuted X^T;
                # consistent with the (p u) row mapping used everywhere)
                xt = io.tile([64, T], f32, tag="xt")
                for q in range(4):
                    pst = psT.tile([64, 512], f32, tag="pst")
                    for i in range(4):
                        u = 4 * q + i
                        nc.tensor.transpose(
                            pst[:, 128 * i:128 * (i + 1)], xsb[:, u, :], ident[:]
                        )
                    nc.vector.tensor_copy(xt[:, 512 * q:512 * (q + 1)], pst[:])

                # G = X^T X  (order-free over row tiles)
                psg = psG.tile([64, 64], f32, tag="psg")
                for u in range(U):
                    nc.tensor.matmul(
                        psg[:], xsb[:, u, :], xsb[:, u, :],
                        start=(u == 0), stop=(u == U - 1),
                    )
                gsb = io.tile([64, 64], f32, tag="gsb")
                nc.scalar.copy(gsb[:], psg[:])

                # out rows: pso[p, :] = out[16p + u, :] = X[16p+u, :] @ G
                osb = io.tile([128, U, D], f32, tag="osb")
                for q in range(4):
                    pso = psO.tile([128, 4, D], f32, tag="pso")
                    for i in range(4):
                        u = 4 * q + i
                        nc.tensor.matmul(
                            pso[:, i, :], xt[:, 128 * u:128 * (u + 1)], gsb[:],
                            start=True, stop=True,
                        )
                    if q % 2 == 0:
                        nc.scalar.copy(osb[:, 4 * q:4 * (q + 1), :], pso[:])
                    else:
                        nc.vector.tensor_copy(osb[:, 4 * q:4 * (q + 1), :], pso[:])

                nc.sync.dma_start(out=yv[:, h], in_=osb[:])

    nc.compile()
    return nc


def _get_nc():
    global _NC
    if _NC is None:
        _NC = _build()
    return _NC


def kernel(x: np.ndarray) -> np.ndarray:
    from concourse.bass_utils import run_bass_kernel_spmd

    assert x.shape == (B, H, T, D), x.shape
    x_flat = np.ascontiguousarray(x.reshape(B * H, T, D), dtype=np.float32)
    in_maps = [
        {"x_shard": np.ascontiguousarray(x_flat[c * HPC:(c + 1) * HPC])}
        for c in range(N_CORES)
    ]
    res = run_bass_kernel_spmd(_get_nc(), in_maps, list(range(N_CORES)))
    out = np.concatenate([res.results[c]["out_shard"] for c in range(N_CORES)], axis=0)
    return out.reshape(B, H, T, D)


# revision 5
# speedup vs baseline: 1.0585x; 1.0585x over previous
"""Trainium2 Bass kernel for nn_MultiHeadAttention_52261162058330.

Reference computes, per (batch, head):
    scores = X @ X.T          # [T, T]
    out    = scores @ X       # [T, D]
with X = x[b, h] of shape [T=2048, D=64], no softmax / no scaling.

Optimizations:
 1. Associativity: out = (X X^T) X = X (X^T X) = X @ G with G = X^T X a
    [64, 64] Gram matrix -> ~32x fewer FLOPs, exact up to summation order.
 2. Split-precision matmuls: X = H + L with H = bf16(X), L = bf16(X - H)
    (covers ~17 mantissa bits).  All matmuls run in bf16 (1 cyc/row on the
    PE + fast weight load vs 4 cyc/row for fp32) accumulating in fp32 PSUM:
      G   = H^T H + H^T L + (H^T L)^T          (drops L^T L ~ 2^-34)
      out = H Gh + H Gl + L Gh                 (G = Gh + Gl split likewise)
    End-to-end rel error ~ 5e-6 vs the fp32 reference.

Sharding: B*H = 32 (batch, head) pairs -> 4 heads per core on 8 cores,
fully independent (no collectives).

Layouts per head (T split as (p u): partition p holds rows 16p..16p+15,
contiguous 4 KiB per partition for DMA):
  hl   [128, 2, 16, 64] bf16 : plane 0 = H, plane 1 = L (each contiguous)
  xth/xtl [128, 8, 128] bf16 : paired transposes; pair s columns are
       X^T of row-tiles u=2s (partitions 0:64) and u=2s+1 (64:128)
  G2h/G2l [128, 128] bf16    : blockdiag(G*, G*) for the paired out-matmul
"""

import numpy as np

N_CORES = 8
B, H, T, D = 2, 16, 2048, 64
HPC = (B * H) // N_CORES  # heads per core
U = T // 128              # 16 row-tiles per head
NP = U // 2               # 8 transpose pairs

_NC = None


def _build():
    import concourse.bacc as bacc
    import concourse.mybir as mybir
    from concourse import tile, masks

    nc = bacc.Bacc(
        trn_type="TRN2", target_bir_lowering=False, debug=False,
        num_devices=N_CORES,
    )
    f32 = mybir.dt.float32
    bf16 = mybir.dt.bfloat16
    sub = mybir.AluOpType.subtract
    x_in = nc.dram_tensor("x_shard", [HPC, T, D], f32, kind="ExternalInput").ap()
    y_out = nc.dram_tensor("out_shard", [HPC, T, D], f32, kind="ExternalOutput").ap()
    xv = x_in.rearrange("h (p u) d -> p h u d", p=128)
    yv = y_out.rearrange("h (p u) d -> p h u d", p=128)

    with tile.TileContext(nc) as tc:
        with (
            tc.tile_pool(name="const", bufs=1) as cpool,
            tc.tile_pool(name="iox", bufs=4) as iox,
            tc.tile_pool(name="io", bufs=2) as io,
            tc.tile_pool(name="psT", bufs=2, space="PSUM") as psT,
            tc.tile_pool(name="psG", bufs=2, space="PSUM") as psG,
            tc.tile_pool(name="psF", bufs=2, space="PSUM") as psF,
            tc.tile_pool(name="psO", bufs=2, space="PSUM") as psO,
        ):
            identb = cpool.tile([128, 128], bf16)
            masks.make_identity(nc, identb[:])
            identf = cpool.tile([64, 64], f32)
            masks.make_identity(nc, identf[:])

            for h in range(HPC):
                xsb = iox.tile([128, U, D], f32, tag="xsb")
                nc.sync.dma_start(out=xsb[:], in_=xv[:, h])

                # H / L split (both planes contiguous per partition)
                hl = io.tile([128, 2, U, D], bf16, tag="hl")
                nc.vector.tensor_copy(hl[:, 0], xsb[:])
                nc.vector.tensor_sub(hl[:, 1], xsb[:], hl[:, 0])

                # Paired transposes of H and L:
                # xth[:, s, p] rows 0:64 = H^T of tile 2s, 64:128 = tile 2s+1
                xth = io.tile([128, NP, 128], bf16, tag="xth")
                xtl = io.tile([128, NP, 128], bf16, tag="xtl")
                for piece, xt in ((0, xth), (1, xtl)):
                    for s in range(NP):
                        pst = psT.tile([128, 128], bf16, tag="pst")
                        nc.tensor.transpose(
                            pst[:], hl[:, piece, 2 * s:2 * s + 2, :], identb[:]
                        )
                        if s % 2 == 0:
                            nc.vector.tensor_copy(xt[:, s, :], pst[:])
                        else:
                            nc.scalar.copy(xt[:, s, :], pst[:])

                # G partials: psg[:, 0] = sum H_u^T H_u, psg[:, 1] = sum H_u^T L_u
                psg = psG.tile([64, 2, D], f32, tag="psg")
                for u in range(U):
                    nc.tensor.matmul(
                        psg[:], hl[:, 0, u, :], hl[:, :, u, :],
                        start=(u == 0), stop=(u == U - 1),
                    )
                shl = io.tile([64, 2, D], f32, tag="shl")
                nc.vector.tensor_copy(shl[:], psg[:])
                # HL^T via PE (fp32, tiny)
                pft = psF.tile([64, D], f32, tag="pf")
                nc.tensor.transpose(pft[:], shl[:, 1, :], identf[:])
                gf = io.tile([64, D], f32, tag="gf")
                nc.vector.tensor_add(gf[:], shl[:, 0, :], shl[:, 1, :])
                nc.vector.tensor_add(gf[:], gf[:], pft[:])

                # G = Gh + Gl split; gcat = [Gh | Gl], gcat2 = [Gl | Gh]
                gcat = io.tile([64, 2, D], bf16, tag="gcat")
                nc.scalar.copy(gcat[:, 0, :], gf[:])
                nc.vector.tensor_sub(gcat[:, 1, :], gf[:], gcat[:, 0, :])
                gcat2 = io.tile([64, 2, D], bf16, tag="gcat2")
                nc.vector.tensor_copy(gcat2[:, 0, :], gcat[:, 1, :])
                nc.vector.tensor_copy(gcat2[:, 1, :], gcat[:, 0, :])
                # transpose(gcat)  -> rows 64:128 = Gl (at partitions 64:128)
                # transpose(gcat2) -> rows 64:128 = Gh (at partitions 64:128)
                pg1 = psF.tile([128, D], bf16, tag="pf")
                nc.tensor.transpose(pg1[:], gcat[:].rearrange("p a b -> p (a b)"), identb[0:64, 0:64])
                pg2 = psF.tile([128, D], bf16, tag="pf")
                nc.tensor.transpose(pg2[:], gcat2[:].rearrange("p a b -> p (a b)"), identb[0:64, 0:64])

                g2h = io.tile([128, 128], bf16, tag="g2h")
                g2l = io.tile([128, 128], bf16, tag="g2l")
                nc.gpsimd.memset(g2h[:], 0.0)
                nc.gpsimd.memset(g2l[:], 0.0)
                nc.vector.tensor_copy(g2h[0:64, 0:64], gcat[:, 0, :])
                nc.scalar.copy(g2h[64:128, 64:128], pg2[64:128, :])
                nc.vector.tensor_copy(g2l[0:64, 0:64], gcat[:, 1, :])
                nc.scalar.copy(g2l[64:128, 64:128], pg1[64:128, :])

                # out pairs: psum[p, 0, :] = out row 16p+2s, [p, 1, :] = 16p+2s+1
                osb = io.tile([128, U, D], f32, tag="osb")
                for s in range(NP):
                    pso = psO.tile([128, 2, D], f32, tag="pso")
                    nc.tensor.matmul(pso[:], xth[:, s, :], g2h[:], start=True, stop=False)
                    nc.tensor.matmul(pso[:], xth[:, s, :], g2l[:], start=False, stop=False)
                    nc.tensor.matmul(pso[:], xtl[:, s, :], g2h[:], start=False, stop=True)
                    if s % 2 == 0:
                        nc.scalar.copy(osb[:, 2 * s:2 * s + 2, :], pso[:])
                    else:
                        nc.vector.tensor_copy(osb[:, 2 * s:2 * s + 2, :], pso[:])

                nc.sync.dma_start(out=yv[:, h], in_=osb[:])

    nc.compile()
    return nc


def _get_nc():
    global _NC
    if _NC is None:
        _NC = _build()
    return _NC


def kernel(x: np.ndarray) -> np.ndarray:
    from concourse.bass_utils import run_bass_kernel_spmd

    assert x.shape == (B, H, T, D), x.shape
    x_flat = np.ascontiguousarray(x.reshape(B * H, T, D), dtype=np.float32)
    in_maps = [
        {"x_shard": np.ascontiguousarray(x_flat[c * HPC:(c + 1) * HPC])}
        for c in range(N_CORES)
    ]
    res = run_bass_kernel_spmd(_get_nc(), in_maps, list(range(N_CORES)))
    out = np.concatenate([res.results[c]["out_shard"] for c in range(N_CORES)], axis=0)
    return out.reshape(B, H, T, D)
